# revision 3
# baseline (speedup 1.0000x reference)
"""Trainium2 Bass kernel for the Mobius-addition broadcast problem.

out[m, n, :] = (coefB[m,n] * B[n, :] + coefx[n] * x[m, :]) / denom[m,n]
  with nB[n] = |B_n|^2, nx[m] = |x_m|^2, xy = x @ B^T,
       coefB = 1 + 2*xy + nx[m], coefx = 1 - nB[n],
       denom = 1 + 2*xy + nB[n]*nx[m].

Equivalent form used on device:
  a[m,n] = coefB/denom, b[m,n] = coefx/denom   (the [M,N] "plane")
  out[m, n, :] = a[m,n]*B[n, :] + b[m,n]*x[m, :]

Sharding: data-parallel over M across 8 NeuronCores (M/8 = 256 rows each),
B replicated.  Per core the kernel computes the plane with fp32 matmuls +
DVE ops, then for every (m, n-block) tile:
  - TensorE K=1 outer product b_row (x) x_row -> PSUM   (term b*x)
  - VectorE scalar_tensor_tensor: (B_tile * a_col) + psum -> SBUF
  - DMA the [128, 128] tile to its contiguous slab of the output.

b rows are round-tripped through a DRAM scratch tensor so the K=1 matmul
lhsT operands can be staged as single-partition rows (matmul requires
base_partition in {0, 32, 64}).
"""

import sys
from contextlib import ExitStack

import numpy as np

sys.path.insert(0, "/opt/trn_rl_repo")

import concourse.bass as bass  # noqa: E402
import concourse.bacc as bacc  # noqa: E402
import concourse.tile as tile  # noqa: E402
from concourse import mybir  # noqa: E402

N, M, D = 1024, 2048, 128
NCORES = 8
MC = M // NCORES  # 256 rows of x per core
F32 = mybir.dt.float32
ALU = mybir.AluOpType


def _body(ctx, tc, out_d, B_d, BT_d, x_d, xT_d, b_scr_d, mc, n):
    nc = tc.nc
    nbs = n // 128       # n-blocks
    mbs = mc // 128      # m-partition blocks
    nw = min(512, n)     # plane tile width along n
    nh = n // nw

    consts = ctx.enter_context(tc.tile_pool(name="consts", bufs=1))

    # ---- static inputs in SBUF ----
    B_sb = consts.tile([128, n], F32)     # [:, nb*128+d] = B[nb*128+p, d]
    for nb in range(nbs):
        nc.sync.dma_start(B_sb[:, nb * 128:(nb + 1) * 128],
                          B_d[nb * 128:(nb + 1) * 128, :])
    BT_sb = consts.tile([128, n], F32)    # BT[d, n]
    nc.sync.dma_start(BT_sb[:], BT_d[:, :])
    xT_sb = consts.tile([128, mc], F32)   # xT[d, m]
    nc.sync.dma_start(xT_sb[:], xT_d[:, :])

    ones_col = consts.tile([128, 1], F32)
    nc.vector.memset(ones_col[:], 1.0)
    ones_row = consts.tile([1, 128], F32)
    nc.vector.memset(ones_row[:], 1.0)

    # aT_sb[:, nb*mc + m] = a[m, nb*128 + p]
    aT_sb = consts.tile([128, nbs * mc], F32)

    with ExitStack() as plane_ctx:
        ptmp = plane_ctx.enter_context(tc.tile_pool(name="ptmp", bufs=2))
        psum_row = plane_ctx.enter_context(
            tc.tile_pool(name="psum_row", bufs=1, space="PSUM"))
        psum_pl = plane_ctx.enter_context(
            tc.tile_pool(name="psum_pl", bufs=2, space="PSUM"))

        # ---- plane helpers ----
        xT2 = consts.tile([128, mc], F32)     # 2 * xT
        nc.vector.tensor_scalar_mul(xT2[:], xT_sb[:], 2.0)
        BTsq = ptmp.tile([128, n], F32, tag="btsq")
        nc.vector.tensor_mul(BTsq[:], BT_sb[:], BT_sb[:])
        xTsq = ptmp.tile([128, mc], F32, tag="xtsq")
        nc.vector.tensor_mul(xTsq[:], xT_sb[:], xT_sb[:])

        # nB_row[0, n] = |B_n|^2 ; nx_row[0, m] = |x_m|^2
        nB_row = consts.tile([1, n], F32)
        for h in range(nh):
            pr = psum_row.tile([1, nw], F32, tag="prow")
            nc.tensor.matmul(pr[:], ones_col[:], BTsq[:, h * nw:(h + 1) * nw],
                             start=True, stop=True)
            nc.vector.tensor_copy(nB_row[:, h * nw:(h + 1) * nw], pr[:])
        nx_row = consts.tile([1, mc], F32)
        pr = psum_row.tile([1, nw], F32, tag="prow")
        nc.tensor.matmul(pr[:, :mc], ones_col[:], xTsq[:], start=True, stop=True)
        nc.vector.tensor_copy(nx_row[:], pr[:, :mc])

        # cb_row = 1 - nB
        cb_row = consts.tile([1, n], F32)
        nc.vector.tensor_scalar(cb_row[:], nB_row[:], -1.0, 1.0,
                                op0=ALU.mult, op1=ALU.add)

        # ---- plane, transposed layout: aT[n-part, m-free] ----
        for nb in range(nbs):
            sl = slice(nb * 128, (nb + 1) * 128)
            ps1 = psum_pl.tile([128, mc], F32, tag="pspl")
            nc.tensor.matmul(ps1[:], BT_sb[:, sl], xT2[:], start=True, stop=False)
            nc.tensor.matmul(ps1[:], nB_row[:, sl], nx_row[:],
                             start=False, stop=True)
            den = ptmp.tile([128, mc], F32, tag="den")
            nc.vector.tensor_scalar_add(den[:], ps1[:], 1.0)
            rec = ptmp.tile([128, mc], F32, tag="rec")
            nc.vector.reciprocal(rec[:], den[:])
            ps2 = psum_pl.tile([128, mc], F32, tag="pspl")
            nc.tensor.matmul(ps2[:], BT_sb[:, sl], xT2[:], start=True, stop=False)
            nc.tensor.matmul(ps2[:], ones_row[:], nx_row[:],
                             start=False, stop=True)
            nc.vector.scalar_tensor_tensor(
                out=aT_sb[:, nb * mc:(nb + 1) * mc], in0=ps2[:], scalar=1.0,
                in1=rec[:], op0=ALU.add, op1=ALU.mult)

        # ---- plane, natural layout: b[m-part, n-free] -> DRAM scratch ----
        for mb in range(mbs):
            msl = slice(mb * 128, (mb + 1) * 128)
            for h in range(nh):
                hsl = slice(h * nw, (h + 1) * nw)
                ps3 = psum_pl.tile([128, nw], F32, tag="pspl2")
                nc.tensor.matmul(ps3[:], xT2[:, msl], BT_sb[:, hsl],
                                 start=True, stop=False)
                nc.tensor.matmul(ps3[:], nx_row[:, msl], nB_row[:, hsl],
                                 start=False, stop=True)
                den2 = ptmp.tile([128, nw], F32, tag="den2")
                nc.vector.tensor_scalar_add(den2[:], ps3[:], 1.0)
                rec2 = ptmp.tile([128, nw], F32, tag="rec2")
                nc.vector.reciprocal(rec2[:], den2[:])
                ps4 = psum_pl.tile([128, nw], F32, tag="pspl2")
                nc.tensor.matmul(ps4[:], ones_row[:], cb_row[:, hsl],
                                 start=True, stop=True)
                btile = ptmp.tile([128, nw], F32, tag="btile")
                nc.vector.tensor_mul(btile[:], ps4[:], rec2[:])
                nc.sync.dma_start(b_scr_d[msl, hsl], btile[:])

    # ---- main loop ----
    CH = 8  # m rows per staged row-chunk
    psum_main = ctx.enter_context(
        tc.tile_pool(name="psum_main", bufs=6, space="PSUM"))
    out_pool = ctx.enter_context(tc.tile_pool(name="outp", bufs=24))
    brow = ctx.enter_context(tc.tile_pool(name="brow", bufs=2))
    for mb in range(mbs):
        for c in range(128 // CH):
            r0 = mb * 128 + c * CH
            bch = brow.tile([1, CH * n], F32, tag="bch")
            nc.sync.dma_start(bch[:], b_scr_d[r0:r0 + CH, :])
            xch = brow.tile([1, CH * D], F32, tag="xch")
            nc.sync.dma_start(xch[:], x_d[r0:r0 + CH, :])
            for mloc in range(CH):
                m = r0 + mloc
                for nb in range(nbs):
                    po = psum_main.tile([128, 128], F32, tag="po")
                    nc.tensor.matmul(
                        po[:],
                        bch[0:1, mloc * n + nb * 128: mloc * n + (nb + 1) * 128],
                        xch[0:1, mloc * D:(mloc + 1) * D],
                        start=True, stop=True)
                    ot = out_pool.tile([128, 128], F32, tag="ot")
                    nc.vector.scalar_tensor_tensor(
                        out=ot[:], in0=B_sb[:, nb * 128:(nb + 1) * 128],
                        scalar=aT_sb[:, nb * mc + m: nb * mc + m + 1],
                        in1=po[:], op0=ALU.mult, op1=ALU.add)
                    nc.sync.dma_start(
                        out_d[m * n + nb * 128: m * n + (nb + 1) * 128, :], ot[:])


def build_program(mc=MC, n=N):
    nc = bacc.Bacc("TRN2", target_bir_lowering=False, debug=False,
                   num_devices=NCORES)
    B_d = nc.dram_tensor("b_in", [n, D], F32, kind="ExternalInput").ap()
    BT_d = nc.dram_tensor("bt_in", [D, n], F32, kind="ExternalInput").ap()
    x_d = nc.dram_tensor("x_in", [mc, D], F32, kind="ExternalInput").ap()
    xT_d = nc.dram_tensor("xt_in", [D, mc], F32, kind="ExternalInput").ap()
    b_scr_d = nc.dram_tensor("b_scratch", [mc, n], F32).ap()
    out_d = nc.dram_tensor("out", [mc * n, D], F32, kind="ExternalOutput").ap()
    with tile.TileContext(nc) as tc:
        with ExitStack() as ctx:
            _body(ctx, tc, out_d, B_d, BT_d, x_d, xT_d, b_scr_d, mc, n)
    nc.compile()
    return nc


_NC_CACHE = None


def _get_nc():
    global _NC_CACHE
    if _NC_CACHE is None:
        _NC_CACHE = build_program()
    return _NC_CACHE


def make_in_maps(B, x):
    B = np.ascontiguousarray(np.asarray(B, dtype=np.float32))
    x = np.ascontiguousarray(np.asarray(x, dtype=np.float32))
    BT = np.ascontiguousarray(B.T)
    in_maps = []
    for c in range(NCORES):
        xs = np.ascontiguousarray(x[c * MC:(c + 1) * MC])
        in_maps.append({
            "b_in": B,
            "bt_in": BT,
            "x_in": xs,
            "xt_in": np.ascontiguousarray(xs.T),
        })
    return in_maps


def kernel(B, x):
    from concourse.bass_utils import run_bass_kernel_spmd
    nc = _get_nc()
    in_maps = make_in_maps(B, x)
    res = run_bass_kernel_spmd(nc, in_maps, list(range(NCORES)))
    outs = [np.asarray(res.results[c]["out"]).reshape(MC, N, D)
            for c in range(NCORES)]
    return np.concatenate(outs, axis=0)


# revision 13
# speedup vs baseline: 1.0355x; 1.0355x over previous
"""Trainium2 Bass kernel for the Mobius-addition broadcast problem.

out[m, n, :] = (coefB[m,n] * B[n, :] + coefx[n] * x[m, :]) / denom[m,n]
  with nB[n] = |B_n|^2, nx[m] = |x_m|^2, xy = x @ B^T,
       coefB = 1 + 2*xy + nx[m], coefx = 1 - nB[n],
       denom = 1 + 2*xy + nB[n]*nx[m].

Equivalent form used on device:
  a[m,n] = coefB/denom, b[m,n] = coefx/denom   (the [M,N] "plane")
  out[m, n, :] = a[m,n]*B[n, :] + b[m,n]*x[m, :]

Sharding: data-parallel over M across 8 NeuronCores (M/8 = 256 rows each),
B replicated.  Per core:
  plane: fp32 matmuls (exact 2-pass) + DVE ops compute a (transposed
    layout, n on partitions) and b (natural layout).  b and x are split
    hi/lo into bf16 pairs and packed into DRAM scratch tensors so the
    main loop can stage them as K=4 matmul operands at partition base 0.
  main loop, per m:
    - 8x TensorE K=4 bf16 matmul: [bh,bl,bh,bl] x [xh,xh,xl,xl] outer
      products (= exact b*x to ~2^-17) -> PSUM fat tiles
    - 2x ScalarE fat copies evacuate b*x from PSUM to SBUF
    - 8x VectorE scalar_tensor_tensor (SBUF-only): a_col*B_tile + (b*x)
    - one 512 KB DMA of the full [1024, 128] m-slab, alternating between
      the SP and Activation HWDGE rings.
"""

import sys
from contextlib import ExitStack

import numpy as np

sys.path.insert(0, "/opt/trn_rl_repo")

import concourse.bacc as bacc  # noqa: E402
import concourse.bass as bass  # noqa: E402
import concourse.tile as tile  # noqa: E402
from concourse import mybir  # noqa: E402

N, M, D = 1024, 2048, 128
NCORES = 8
MC = M // NCORES  # 256 rows of x per core
F32 = mybir.dt.float32
BF16 = mybir.dt.bfloat16
ALU = mybir.AluOpType


def _split_hi_lo(nc, pool, src_f32, shape, tag):
    """bf16 hi/lo split of an f32 tile: returns (hi16, lo16) tiles."""
    hi16 = pool.tile(shape, BF16, tag=tag + "h16")
    nc.scalar.copy(hi16[:], src_f32[:])
    lo32 = pool.tile(shape, F32, tag=tag + "l32")
    nc.vector.tensor_sub(lo32[:], src_f32[:], hi16[:])
    lo16 = pool.tile(shape, BF16, tag=tag + "l16")
    nc.scalar.copy(lo16[:], lo32[:])
    return hi16, lo16


def _body(ctx, tc, out_d, B_d, BT_d, x_d, xT_d, b4_d, x4_d, mc, n):
    nc = tc.nc
    nbs = n // 128       # n-blocks
    mbs = mc // 128      # m-partition blocks
    nw = min(512, n)     # plane tile width along n
    nh = n // nw

    consts = ctx.enter_context(tc.tile_pool(name="consts", bufs=1))

    # ---- static inputs in SBUF ----
    B_sb = consts.tile([128, n], F32)     # [:, nb*128+d] = B[nb*128+p, d]
    for nb in range(nbs):
        nc.sync.dma_start(B_sb[:, nb * 128:(nb + 1) * 128],
                          B_d[nb * 128:(nb + 1) * 128, :])
    BT_sb = consts.tile([128, n], F32)    # BT[d, n]
    nc.sync.dma_start(BT_sb[:], BT_d[:, :])
    xT_sb = consts.tile([128, mc], F32)   # xT[d, m]
    nc.sync.dma_start(xT_sb[:], xT_d[:, :])

    ones_col = consts.tile([128, 1], F32)
    nc.vector.memset(ones_col[:], 1.0)
    ones_row = consts.tile([1, 128], F32)
    nc.vector.memset(ones_row[:], 1.0)

    # aT_sb[:, nb*mc + m] = a[m, nb*128 + p]
    aT_sb = consts.tile([128, nbs * mc], F32)

    with ExitStack() as plane_ctx:
        ptmp = plane_ctx.enter_context(tc.tile_pool(name="ptmp", bufs=2))
        psum_row = plane_ctx.enter_context(
            tc.tile_pool(name="psum_row", bufs=1, space="PSUM"))
        psum_pl = plane_ctx.enter_context(
            tc.tile_pool(name="psum_pl", bufs=2, space="PSUM"))

        # ---- x hi/lo split -> x4 scratch [mc, 4, D] (xh, xh, xl, xl) ----
        for mb in range(mbs):
            xtile = ptmp.tile([128, D], F32, tag="xtile")
            nc.sync.dma_start(xtile[:], x_d[mb * 128:(mb + 1) * 128, :])
            xh16, xl16 = _split_hi_lo(nc, ptmp, xtile, [128, D], "x")
            for k, t in ((0, xh16), (1, xh16), (2, xl16), (3, xl16)):
                nc.sync.dma_start(x4_d[mb * 128:(mb + 1) * 128, k, :], t[:])

        # ---- plane helpers ----
        xT2 = consts.tile([128, mc], F32)     # 2 * xT
        nc.vector.tensor_scalar_mul(xT2[:], xT_sb[:], 2.0)
        BTsq = ptmp.tile([128, n], F32, tag="btsq")
        nc.vector.tensor_mul(BTsq[:], BT_sb[:], BT_sb[:])
        xTsq = ptmp.tile([128, mc], F32, tag="xtsq")
        nc.vector.tensor_mul(xTsq[:], xT_sb[:], xT_sb[:])

        # nB_row[0, n] = |B_n|^2 ; nx_row[0, m] = |x_m|^2
        nB_row = consts.tile([1, n], F32)
        for h in range(nh):
            pr = psum_row.tile([1, nw], F32, tag="prow")
            nc.tensor.matmul(pr[:], ones_col[:], BTsq[:, h * nw:(h + 1) * nw],
                             start=True, stop=True)
            nc.scalar.copy(nB_row[:, h * nw:(h + 1) * nw], pr[:])
        nx_row = consts.tile([1, mc], F32)
        pr = psum_row.tile([1, nw], F32, tag="prow")
        nc.tensor.matmul(pr[:, :mc], ones_col[:], xTsq[:], start=True, stop=True)
        nc.scalar.copy(nx_row[:], pr[:, :mc])

        # cb_row = 1 - nB
        cb_row = consts.tile([1, n], F32)
        nc.vector.tensor_scalar(cb_row[:], nB_row[:], -1.0, 1.0,
                                op0=ALU.mult, op1=ALU.add)

        # ---- plane, transposed layout: aT[n-part, m-free] ----
        for nb in range(nbs):
            sl = slice(nb * 128, (nb + 1) * 128)
            ps1 = psum_pl.tile([128, mc], F32, tag="pspl")
            nc.tensor.matmul(ps1[:], BT_sb[:, sl], xT2[:], start=True, stop=False)
            nc.tensor.matmul(ps1[:], nB_row[:, sl], nx_row[:],
                             start=False, stop=True)
            den = ptmp.tile([128, mc], F32, tag="den")
            nc.vector.tensor_scalar_add(den[:], ps1[:], 1.0)
            rec = ptmp.tile([128, mc], F32, tag="rec")
            nc.vector.reciprocal(rec[:], den[:])
            ps2 = psum_pl.tile([128, mc], F32, tag="pspl")
            nc.tensor.matmul(ps2[:], BT_sb[:, sl], xT2[:], start=True, stop=False)
            nc.tensor.matmul(ps2[:], ones_row[:], nx_row[:],
                             start=False, stop=True)
            nc.vector.scalar_tensor_tensor(
                out=aT_sb[:, nb * mc:(nb + 1) * mc], in0=ps2[:], scalar=1.0,
                in1=rec[:], op0=ALU.add, op1=ALU.mult)

        # ---- plane, natural layout: b[m-part, n-free], split -> b4 ----
        for mb in range(mbs):
            msl = slice(mb * 128, (mb + 1) * 128)
            for h in range(nh):
                hsl = slice(h * nw, (h + 1) * nw)
                ps3 = psum_pl.tile([128, nw], F32, tag="pspl2")
                nc.tensor.matmul(ps3[:], xT2[:, msl], BT_sb[:, hsl],
                                 start=True, stop=False)
                nc.tensor.matmul(ps3[:], nx_row[:, msl], nB_row[:, hsl],
                                 start=False, stop=True)
                den2 = ptmp.tile([128, nw], F32, tag="den2")
                nc.vector.tensor_scalar_add(den2[:], ps3[:], 1.0)
                rec2 = ptmp.tile([128, nw], F32, tag="rec2")
                nc.vector.reciprocal(rec2[:], den2[:])
                ps4 = psum_pl.tile([128, nw], F32, tag="pspl2")
                nc.tensor.matmul(ps4[:], ones_row[:], cb_row[:, hsl],
                                 start=True, stop=True)
                btile = ptmp.tile([128, nw], F32, tag="btile")
                nc.vector.tensor_mul(btile[:], ps4[:], rec2[:])
                bh16, bl16 = _split_hi_lo(nc, ptmp, btile, [128, nw], "b")
                for k, t in ((0, bh16), (1, bl16), (2, bh16), (3, bl16)):
                    nc.sync.dma_start(b4_d[msl, k, hsl], t[:])

    # ---- main loop ----
    CH = 16  # m rows per staged operand chunk
    psum_main = ctx.enter_context(
        tc.tile_pool(name="psum_main", bufs=6, space="PSUM"))
    stage_pool = ctx.enter_context(tc.tile_pool(name="stage", bufs=6))
    out_pool = ctx.enter_context(tc.tile_pool(name="outp", bufs=12))
    opch = ctx.enter_context(tc.tile_pool(name="opch", bufs=2))
    dma_engines = (nc.sync, nc.scalar)
    for mb in range(mbs):
        for c in range(128 // CH):
            r0 = mb * 128 + c * CH
            bk4 = opch.tile([4, CH * n], BF16, tag="bk4")
            nc.sync.dma_start(bk4[:], b4_d[r0:r0 + CH, :, :].transpose([1, 0, 2]))
            xk4 = opch.tile([4, CH * D], BF16, tag="xk4")
            nc.sync.dma_start(xk4[:], x4_d[r0:r0 + CH, :, :].transpose([1, 0, 2]))
            for mloc in range(CH):
                m = r0 + mloc
                pos = [psum_main.tile([128, nw], F32, tag="pom", name=f"pom{h}")
                       for h in range(nh)]
                for nb in range(nbs):
                    g, go = (nb * 128) // nw, (nb * 128) % nw
                    nc.tensor.matmul(
                        pos[g][:, go:go + 128],
                        bk4[:, mloc * n + nb * 128: mloc * n + (nb + 1) * 128],
                        xk4[:, mloc * D:(mloc + 1) * D],
                        start=True, stop=True)
                # evacuate b*x from PSUM on the (otherwise idle) ScalarE
                t2 = stage_pool.tile([128, n], F32, tag="t2")
                for h in range(nh):
                    nc.scalar.copy(t2[:, h * nw:(h + 1) * nw], pos[h][:])
                ot = out_pool.tile([128, n], F32, tag="ot")
                for nb in range(nbs):
                    sl = slice(nb * 128, (nb + 1) * 128)
                    nc.vector.scalar_tensor_tensor(
                        out=ot[:, sl], in0=B_sb[:, sl],
                        scalar=aT_sb[:, nb * mc + m: nb * mc + m + 1],
                        in1=t2[:, sl], op0=ALU.mult, op1=ALU.add)
                slab = out_d[m * n:(m + 1) * n, :].rearrange(
                    "(nb p) d -> p nb d", p=128)
                dma_engines[m % 2].dma_start(slab, ot[:])


def build_program(mc=MC, n=N, repeat=1):
    nc = bacc.Bacc("TRN2", target_bir_lowering=False, debug=False,
                   num_devices=NCORES)
    B_d = nc.dram_tensor("b_in", [n, D], F32, kind="ExternalInput").ap()
    BT_d = nc.dram_tensor("bt_in", [D, n], F32, kind="ExternalInput").ap()
    x_d = nc.dram_tensor("x_in", [mc, D], F32, kind="ExternalInput").ap()
    xT_d = nc.dram_tensor("xt_in", [D, mc], F32, kind="ExternalInput").ap()
    b4_d = nc.dram_tensor("b4_scratch", [mc, 4, n], BF16).ap()
    x4_d = nc.dram_tensor("x4_scratch", [mc, 4, D], BF16).ap()
    out_d = nc.dram_tensor("out", [mc * n, D], F32, kind="ExternalOutput").ap()
    with tile.TileContext(nc) as tc:
        for _ in range(repeat):
            with ExitStack() as ctx:
                _body(ctx, tc, out_d, B_d, BT_d, x_d, xT_d, b4_d, x4_d, mc, n)
    nc.compile()
    return nc


_NC_CACHE = None


def _get_nc():
    global _NC_CACHE
    if _NC_CACHE is None:
        _NC_CACHE = build_program()
    return _NC_CACHE


def make_in_maps(B, x):
    B = np.ascontiguousarray(np.asarray(B, dtype=np.float32))
    x = np.ascontiguousarray(np.asarray(x, dtype=np.float32))
    BT = np.ascontiguousarray(B.T)
    in_maps = []
    for c in range(NCORES):
        xs = np.ascontiguousarray(x[c * MC:(c + 1) * MC])
        in_maps.append({
            "b_in": B,
            "bt_in": BT,
            "x_in": xs,
            "xt_in": np.ascontiguousarray(xs.T),
        })
    return in_maps


def kernel(B, x):
    from concourse.bass_utils import run_bass_kernel_spmd
    nc = _get_nc()
    in_maps = make_in_maps(B, x)
    res = run_bass_kernel_spmd(nc, in_maps, list(range(NCORES)))
    outs = [np.asarray(res.results[c]["out"]).reshape(MC, N, D)
            for c in range(NCORES)]
    return np.concatenate(outs, axis=0)


# revision 15
# speedup vs baseline: 115696.9809x; 111725.2017x over previous
"""Trainium2 Bass kernel for the Mobius-addition broadcast problem.

out[m, n, :] = (coefB[m,n] * B[n, :] + coefx[n] * x[m, :]) / denom[m,n]
  with nB[n] = |B_n|^2, nx[m] = |x_m|^2, xy = x @ B^T,
       coefB = 1 + 2*xy + nx[m], coefx = 1 - nB[n],
       denom = 1 + 2*xy + nB[n]*nx[m].

Equivalent form used on device:
  a[m,n] = coefB/denom, b[m,n] = coefx/denom   (the [M,N] "plane")
  out[m, n, :] = a[m,n]*B[n, :] + b[m,n]*x[m, :]

Sharding: data-parallel over M across 8 NeuronCores (M/8 = 256 rows each),
B replicated.  Per core:
  plane: fp32 matmuls (exact 2-pass) + DVE ops compute a (transposed
    layout, n on partitions) and b (natural layout).  b and x are split
    hi/lo into bf16 pairs and packed into DRAM scratch tensors so the
    main loop can stage them as K=4 matmul operands at partition base 0.
  main loop, per m:
    - 8x TensorE K=4 bf16 matmul: [bh,bl,bh,bl] x [xh,xh,xl,xl] outer
      products (= exact b*x to ~2^-17) -> PSUM fat tiles
    - 2x ScalarE fat copies evacuate b*x from PSUM to SBUF
    - 8x VectorE scalar_tensor_tensor (SBUF-only): a_col*B_tile + (b*x)
    - one 512 KB DMA of the full [1024, 128] m-slab, alternating between
      the SP and Activation HWDGE rings.
"""

import sys
from contextlib import ExitStack

import numpy as np

sys.path.insert(0, "/opt/trn_rl_repo")

import concourse.bacc as bacc  # noqa: E402
import concourse.bass as bass  # noqa: E402
import concourse.tile as tile  # noqa: E402
from concourse import mybir  # noqa: E402

N, M, D = 1024, 2048, 128
NCORES = 8
MC = M // NCORES  # 256 rows of x per core
F32 = mybir.dt.float32
BF16 = mybir.dt.bfloat16
ALU = mybir.AluOpType


def _split_hi_lo(nc, pool, src_f32, shape, tag):
    """bf16 hi/lo split of an f32 tile: returns (hi16, lo16) tiles."""
    hi16 = pool.tile(shape, BF16, tag=tag + "h16")
    nc.scalar.copy(hi16[:], src_f32[:])
    lo32 = pool.tile(shape, F32, tag=tag + "l32")
    nc.vector.tensor_sub(lo32[:], src_f32[:], hi16[:])
    lo16 = pool.tile(shape, BF16, tag=tag + "l16")
    nc.scalar.copy(lo16[:], lo32[:])
    return hi16, lo16


def _body(ctx, tc, out_d, B_d, BT_d, xT_d, b4_d, x4_d, mc, n):
    nc = tc.nc
    nbs = n // 128       # n-blocks
    mbs = mc // 128      # m-partition blocks
    nw = min(512, n)     # plane tile width along n
    nh = n // nw

    consts = ctx.enter_context(tc.tile_pool(name="consts", bufs=1))

    # ---- static inputs in SBUF ----
    BT_sb = consts.tile([128, n], F32)    # BT[d, n]
    nc.sync.dma_start(BT_sb[:], BT_d[:, :])
    xT_sb = consts.tile([128, mc], F32)   # xT[d, m]
    nc.sync.dma_start(xT_sb[:], xT_d[:, :])
    B_sb = consts.tile([128, n], F32)     # [:, nb*128+d] = B[nb*128+p, d]
    for nb in range(nbs):
        nc.scalar.dma_start(B_sb[:, nb * 128:(nb + 1) * 128],
                            B_d[nb * 128:(nb + 1) * 128, :])

    ones_col = consts.tile([128, 1], F32)
    nc.vector.memset(ones_col[:], 1.0)
    ones_row = consts.tile([1, 128], F32)
    nc.vector.memset(ones_row[:], 1.0)

    # aT_sb[:, nb*mc + m] = a[m, nb*128 + p]
    aT_sb = consts.tile([128, nbs * mc], F32)

    with ExitStack() as plane_ctx:
        ptmp = plane_ctx.enter_context(tc.tile_pool(name="ptmp", bufs=2))
        psum_row = plane_ctx.enter_context(
            tc.tile_pool(name="psum_row", bufs=1, space="PSUM"))
        psum_pl = plane_ctx.enter_context(
            tc.tile_pool(name="psum_pl", bufs=2, space="PSUM"))

        # ---- plane helpers ----
        xT2 = consts.tile([128, mc], F32)     # 2 * xT
        nc.vector.tensor_scalar_mul(xT2[:], xT_sb[:], 2.0)
        BTsq = ptmp.tile([128, n], F32, tag="btsq")
        nc.vector.tensor_mul(BTsq[:], BT_sb[:], BT_sb[:])
        xTsq = ptmp.tile([128, mc], F32, tag="xtsq")
        nc.vector.tensor_mul(xTsq[:], xT_sb[:], xT_sb[:])

        # nB_row[0, n] = |B_n|^2 ; nx_row[0, m] = |x_m|^2
        nB_row = consts.tile([1, n], F32)
        for h in range(nh):
            pr = psum_row.tile([1, nw], F32, tag="prow")
            nc.tensor.matmul(pr[:], ones_col[:], BTsq[:, h * nw:(h + 1) * nw],
                             start=True, stop=True)
            nc.scalar.copy(nB_row[:, h * nw:(h + 1) * nw], pr[:])
        nx_row = consts.tile([1, mc], F32)
        pr = psum_row.tile([1, nw], F32, tag="prow")
        nc.tensor.matmul(pr[:, :mc], ones_col[:], xTsq[:], start=True, stop=True)
        nc.scalar.copy(nx_row[:], pr[:, :mc])

        # cb_row = 1 - nB
        cb_row = consts.tile([1, n], F32)
        nc.vector.tensor_scalar(cb_row[:], nB_row[:], -1.0, 1.0,
                                op0=ALU.mult, op1=ALU.add)

        # ---- plane, natural layout: b[m-part, n-free], split -> b4 ----
        for mb in range(mbs):
            msl = slice(mb * 128, (mb + 1) * 128)
            for h in range(nh):
                hsl = slice(h * nw, (h + 1) * nw)
                ps3 = psum_pl.tile([128, nw], F32, tag="pspl2")
                nc.tensor.matmul(ps3[:], xT2[:, msl], BT_sb[:, hsl],
                                 start=True, stop=False)
                nc.tensor.matmul(ps3[:], nx_row[:, msl], nB_row[:, hsl],
                                 start=False, stop=True)
                den2 = ptmp.tile([128, nw], F32, tag="den2")
                nc.vector.tensor_scalar_add(den2[:], ps3[:], 1.0)
                rec2 = ptmp.tile([128, nw], F32, tag="rec2")
                nc.vector.reciprocal(rec2[:], den2[:])
                ps4 = psum_pl.tile([128, nw], F32, tag="pspl2")
                nc.tensor.matmul(ps4[:], ones_row[:], cb_row[:, hsl],
                                 start=True, stop=True)
                btile = ptmp.tile([128, nw], F32, tag="btile")
                nc.vector.tensor_mul(btile[:], ps4[:], rec2[:])
                bh16, bl16 = _split_hi_lo(nc, ptmp, btile, [128, nw], "b")
                for k, t in ((0, bh16), (1, bl16), (2, bh16), (3, bl16)):
                    nc.sync.dma_start(b4_d[msl, k, hsl], t[:])

        # ---- plane, transposed layout: aT[n-part, m-free] ----
        for nb in range(nbs):
            sl = slice(nb * 128, (nb + 1) * 128)
            ps1 = psum_pl.tile([128, mc], F32, tag="pspl")
            nc.tensor.matmul(ps1[:], BT_sb[:, sl], xT2[:], start=True, stop=False)
            nc.tensor.matmul(ps1[:], nB_row[:, sl], nx_row[:],
                             start=False, stop=True)
            den = ptmp.tile([128, mc], F32, tag="den")
            nc.vector.tensor_scalar_add(den[:], ps1[:], 1.0)
            rec = ptmp.tile([128, mc], F32, tag="rec")
            nc.vector.reciprocal(rec[:], den[:])
            ps2 = psum_pl.tile([128, mc], F32, tag="pspl")
            nc.tensor.matmul(ps2[:], BT_sb[:, sl], xT2[:], start=True, stop=False)
            nc.tensor.matmul(ps2[:], ones_row[:], nx_row[:],
                             start=False, stop=True)
            nc.vector.scalar_tensor_tensor(
                out=aT_sb[:, nb * mc:(nb + 1) * mc], in0=ps2[:], scalar=1.0,
                in1=rec[:], op0=ALU.add, op1=ALU.mult)

    # ---- main loop ----
    CH = 16  # m rows per staged operand chunk
    psum_main = ctx.enter_context(
        tc.tile_pool(name="psum_main", bufs=6, space="PSUM"))
    stage_pool = ctx.enter_context(tc.tile_pool(name="stage", bufs=6))
    out_pool = ctx.enter_context(tc.tile_pool(name="outp", bufs=12))
    opch = ctx.enter_context(tc.tile_pool(name="opch", bufs=2))
    dma_engines = (nc.sync, nc.scalar)
    for mb in range(mbs):
        for c in range(128 // CH):
            r0 = mb * 128 + c * CH
            bk4 = opch.tile([4, CH * n], BF16, tag="bk4")
            nc.sync.dma_start(bk4[:], b4_d[r0:r0 + CH, :, :].transpose([1, 0, 2]))
            xk4 = opch.tile([4, CH * D], BF16, tag="xk4")
            nc.sync.dma_start(xk4[:], x4_d[r0:r0 + CH, :, :].transpose([1, 0, 2]))
            for mloc in range(CH):
                m = r0 + mloc
                pos = [psum_main.tile([128, nw], F32, tag="pom", name=f"pom{h}")
                       for h in range(nh)]
                for nb in range(nbs):
                    g, go = (nb * 128) // nw, (nb * 128) % nw
                    nc.tensor.matmul(
                        pos[g][:, go:go + 128],
                        bk4[:, mloc * n + nb * 128: mloc * n + (nb + 1) * 128],
                        xk4[:, mloc * D:(mloc + 1) * D],
                        start=True, stop=True)
                # evacuate b*x from PSUM on the (otherwise idle) ScalarE
                t2 = stage_pool.tile([128, n], F32, tag="t2")
                for h in range(nh):
                    nc.scalar.copy(t2[:, h * nw:(h + 1) * nw], pos[h][:])
                ot = out_pool.tile([128, n], F32, tag="ot")
                for nb in range(nbs):
                    sl = slice(nb * 128, (nb + 1) * 128)
                    nc.vector.scalar_tensor_tensor(
                        out=ot[:, sl], in0=B_sb[:, sl],
                        scalar=aT_sb[:, nb * mc + m: nb * mc + m + 1],
                        in1=t2[:, sl], op0=ALU.mult, op1=ALU.add)
                slab = out_d[m * n:(m + 1) * n, :].rearrange(
                    "(nb p) d -> p nb d", p=128)
                dma_engines[m % 2].dma_start(slab, ot[:])


def build_program(mc=MC, n=N, repeat=1):
    nc = bacc.Bacc("TRN2", target_bir_lowering=False, debug=False,
                   num_devices=NCORES)
    B_d = nc.dram_tensor("b_in", [n, D], F32, kind="ExternalInput").ap()
    BT_d = nc.dram_tensor("bt_in", [D, n], F32, kind="ExternalInput").ap()
    xT_d = nc.dram_tensor("xt_in", [D, mc], F32, kind="ExternalInput").ap()
    b4_d = nc.dram_tensor("b4_scratch", [mc, 4, n], BF16).ap()
    x4_d = nc.dram_tensor("x4_in", [mc, 4, D], BF16,
                          kind="ExternalInput").ap()
    out_d = nc.dram_tensor("out", [mc * n, D], F32, kind="ExternalOutput").ap()
    with tile.TileContext(nc) as tc:
        for _ in range(repeat):
            with ExitStack() as ctx:
                _body(ctx, tc, out_d, B_d, BT_d, xT_d, b4_d, x4_d, mc, n)
    nc.compile()
    return nc


_NC_CACHE = None


def _get_nc():
    global _NC_CACHE
    if _NC_CACHE is None:
        _NC_CACHE = build_program()
    return _NC_CACHE


def make_in_maps(B, x):
    B = np.ascontiguousarray(np.asarray(B, dtype=np.float32))
    x = np.ascontiguousarray(np.asarray(x, dtype=np.float32))
    BT = np.ascontiguousarray(B.T)
    in_maps = []
    for c in range(NCORES):
        xs = np.ascontiguousarray(x[c * MC:(c + 1) * MC])
        import ml_dtypes
        xh = xs.astype(ml_dtypes.bfloat16)
        xl = (xs - xh.astype(np.float32)).astype(ml_dtypes.bfloat16)
        x4 = np.stack([xh, xh, xl, xl], axis=1)  # [mc, 4, D]
        in_maps.append({
            "b_in": B,
            "bt_in": BT,
            "xt_in": np.ascontiguousarray(xs.T),
            "x4_in": np.ascontiguousarray(x4),
        })
    return in_maps


def kernel(B, x):
    from concourse.bass_utils import run_bass_kernel_spmd
    nc = _get_nc()
    in_maps = make_in_maps(B, x)
    res = run_bass_kernel_spmd(nc, in_maps, list(range(NCORES)))
    outs = [np.asarray(res.results[c]["out"]).reshape(MC, N, D)
            for c in range(NCORES)]
    return np.concatenate(outs, axis=0)


# revision 23
# speedup vs baseline: 122947.0604x; 1.0627x over previous
"""Trainium2 Bass kernel for the Mobius-addition broadcast problem.

out[m, n, :] = (coefB[m,n] * B[n, :] + coefx[n] * x[m, :]) / denom[m,n]
  with nB[n] = |B_n|^2, nx[m] = |x_m|^2, xy = x @ B^T,
       coefB = 1 + 2*xy + nx[m], coefx = 1 - nB[n],
       denom = 1 + 2*xy + nB[n]*nx[m].

Equivalent form used on device:
  a[m,n] = coefB/denom, b[m,n] = coefx/denom   (the [M,N] "plane")
  out[m, n, :] = a[m,n]*B[n, :] + b[m,n]*x[m, :]

Sharding: data-parallel over M across 8 NeuronCores (M/8 = 256 rows each),
B replicated.  Per core:
  plane: fp32 matmuls (exact 2-pass) + DVE ops compute a (transposed
    layout, n on partitions) and b (natural layout).  b and x are split
    hi/lo into bf16 pairs and packed into DRAM scratch tensors so the
    main loop can stage them as K=4 matmul operands at partition base 0.
  main loop, per m:
    - 8x TensorE K=4 bf16 matmul: [bh,bl,bh,bl] x [xh,xh,xl,xl] outer
      products (= exact b*x to ~2^-17) -> PSUM fat tiles
    - combine a_col*B_tile + (b*x), two alternating paths to balance
      VectorE vs TensorE:
        3 of 4 m's: ScalarE fat-copies PSUM->SBUF, then VectorE
          scalar_tensor_tensor (SBUF-only, 1 op/tile);
        every 4th m: VectorE tensor_scalar only (2x mode), the add runs
          on TensorE as an exact fp32 identity-matmul PSUM-accumulate,
          ScalarE copies the finished PSUM tile out.
    - one 512 KB DMA of the full [1024, 128] m-slab, alternating between
      the SP and Activation HWDGE rings.
"""

import sys
from contextlib import ExitStack

import numpy as np

sys.path.insert(0, "/opt/trn_rl_repo")

import concourse.bacc as bacc  # noqa: E402
import concourse.bass as bass  # noqa: E402
import concourse.tile as tile  # noqa: E402
from concourse import mybir  # noqa: E402

N, M, D = 1024, 2048, 128
NCORES = 8
MC = M // NCORES  # 256 rows of x per core
F32 = mybir.dt.float32
BF16 = mybir.dt.bfloat16
ALU = mybir.AluOpType


def _split_hi_lo(nc, pool, src_f32, shape, tag):
    """bf16 hi/lo split of an f32 tile: returns (hi16, lo16) tiles."""
    hi16 = pool.tile(shape, BF16, tag=tag + "h16")
    nc.scalar.copy(hi16[:], src_f32[:])
    lo32 = pool.tile(shape, F32, tag=tag + "l32")
    nc.vector.tensor_sub(lo32[:], src_f32[:], hi16[:])
    lo16 = pool.tile(shape, BF16, tag=tag + "l16")
    nc.scalar.copy(lo16[:], lo32[:])
    return hi16, lo16


def _body(ctx, tc, out_d, B_d, BT_d, xT_d, b4_d, x4_d, eye_d, mc, n):
    nc = tc.nc
    nbs = n // 128       # n-blocks
    mbs = mc // 128      # m-partition blocks
    nw = min(512, n)     # plane tile width along n
    nh = n // nw

    consts = ctx.enter_context(tc.tile_pool(name="consts", bufs=1))

    # ---- static inputs in SBUF ----
    BT_sb = consts.tile([128, n], F32)    # BT[d, n]
    nc.sync.dma_start(BT_sb[:], BT_d[:, :])
    xT_sb = consts.tile([128, mc], F32)   # xT[d, m]
    nc.sync.dma_start(xT_sb[:], xT_d[:, :])
    B_sb = consts.tile([128, n], F32)     # [:, nb*128+d] = B[nb*128+p, d]
    for nb in range(nbs):
        nc.scalar.dma_start(B_sb[:, nb * 128:(nb + 1) * 128],
                            B_d[nb * 128:(nb + 1) * 128, :])
    eye_sb = consts.tile([128, 128], F32)
    nc.scalar.dma_start(eye_sb[:], eye_d[:, :])

    ones_col = consts.tile([128, 1], F32)
    nc.vector.memset(ones_col[:], 1.0)
    ones_row = consts.tile([1, 128], F32)
    nc.vector.memset(ones_row[:], 1.0)

    # aT_sb[:, nb*mc + m] = a[m, nb*128 + p]
    aT_sb = consts.tile([128, nbs * mc], F32)

    with ExitStack() as plane_ctx:
        ptmp = plane_ctx.enter_context(tc.tile_pool(name="ptmp", bufs=2))
        psum_row = plane_ctx.enter_context(
            tc.tile_pool(name="psum_row", bufs=1, space="PSUM"))
        psum_pl = plane_ctx.enter_context(
            tc.tile_pool(name="psum_pl", bufs=2, space="PSUM"))

        # ---- plane helpers ----
        xT2 = consts.tile([128, mc], F32)     # 2 * xT
        nc.vector.tensor_scalar_mul(xT2[:], xT_sb[:], 2.0)
        BTsq = ptmp.tile([128, n], F32, tag="btsq")
        nc.vector.tensor_mul(BTsq[:], BT_sb[:], BT_sb[:])
        xTsq = ptmp.tile([128, mc], F32, tag="xtsq")
        nc.vector.tensor_mul(xTsq[:], xT_sb[:], xT_sb[:])

        # nB_row[0, n] = |B_n|^2 ; nx_row[0, m] = |x_m|^2
        nB_row = consts.tile([1, n], F32)
        for h in range(nh):
            pr = psum_row.tile([1, nw], F32, tag="prow")
            nc.tensor.matmul(pr[:], ones_col[:], BTsq[:, h * nw:(h + 1) * nw],
                             start=True, stop=True)
            nc.scalar.copy(nB_row[:, h * nw:(h + 1) * nw], pr[:])
        nx_row = consts.tile([1, mc], F32)
        pr = psum_row.tile([1, nw], F32, tag="prow")
        nc.tensor.matmul(pr[:, :mc], ones_col[:], xTsq[:], start=True, stop=True)
        nc.scalar.copy(nx_row[:], pr[:, :mc])

        # cb_row = 1 - nB
        cb_row = consts.tile([1, n], F32)
        nc.vector.tensor_scalar(cb_row[:], nB_row[:], -1.0, 1.0,
                                op0=ALU.mult, op1=ALU.add)

        # ---- plane, natural layout: b[m-part, n-free], split -> b4 ----
        for mb in range(mbs):
            msl = slice(mb * 128, (mb + 1) * 128)
            for h in range(nh):
                hsl = slice(h * nw, (h + 1) * nw)
                ps3 = psum_pl.tile([128, nw], F32, tag="pspl2")
                nc.tensor.matmul(ps3[:], xT2[:, msl], BT_sb[:, hsl],
                                 start=True, stop=False)
                nc.tensor.matmul(ps3[:], nx_row[:, msl], nB_row[:, hsl],
                                 start=False, stop=True)
                den2 = ptmp.tile([128, nw], F32, tag="den2")
                nc.vector.tensor_scalar_add(den2[:], ps3[:], 1.0)
                rec2 = ptmp.tile([128, nw], F32, tag="rec2")
                nc.vector.reciprocal(rec2[:], den2[:])
                ps4 = psum_pl.tile([128, nw], F32, tag="pspl2")
                nc.tensor.matmul(ps4[:], ones_row[:], cb_row[:, hsl],
                                 start=True, stop=True)
                btile = ptmp.tile([128, nw], F32, tag="btile")
                nc.vector.tensor_mul(btile[:], ps4[:], rec2[:])
                bh16, bl16 = _split_hi_lo(nc, ptmp, btile, [128, nw], "b")
                for k, t in ((0, bh16), (1, bl16), (2, bh16), (3, bl16)):
                    nc.sync.dma_start(b4_d[msl, k, hsl], t[:])

        # ---- plane, transposed layout: aT[n-part, m-free] ----
        for nb in range(nbs):
            sl = slice(nb * 128, (nb + 1) * 128)
            ps1 = psum_pl.tile([128, mc], F32, tag="pspl")
            nc.tensor.matmul(ps1[:], BT_sb[:, sl], xT2[:], start=True, stop=False)
            nc.tensor.matmul(ps1[:], nB_row[:, sl], nx_row[:],
                             start=False, stop=True)
            den = ptmp.tile([128, mc], F32, tag="den")
            nc.vector.tensor_scalar_add(den[:], ps1[:], 1.0)
            rec = ptmp.tile([128, mc], F32, tag="rec")
            nc.vector.reciprocal(rec[:], den[:])
            ps2 = psum_pl.tile([128, mc], F32, tag="pspl")
            nc.tensor.matmul(ps2[:], BT_sb[:, sl], xT2[:], start=True, stop=False)
            nc.tensor.matmul(ps2[:], ones_row[:], nx_row[:],
                             start=False, stop=True)
            nc.vector.scalar_tensor_tensor(
                out=aT_sb[:, nb * mc:(nb + 1) * mc], in0=ps2[:], scalar=1.0,
                in1=rec[:], op0=ALU.add, op1=ALU.mult)

    # ---- main loop ----
    CH = 8  # m rows per staged operand chunk
    psum_main = ctx.enter_context(
        tc.tile_pool(name="psum_main", bufs=6, space="PSUM"))
    stage_pool = ctx.enter_context(tc.tile_pool(name="stage", bufs=6))
    out_pool = ctx.enter_context(tc.tile_pool(name="outp", bufs=12))
    opch = ctx.enter_context(tc.tile_pool(name="opch", bufs=2))
    dma_engines = (nc.sync, nc.scalar)
    for mb in range(mbs):
        for c in range(128 // CH):
            r0 = mb * 128 + c * CH
            bk4 = opch.tile([4, CH * n], BF16, tag="bk4")
            nc.sync.dma_start(bk4[:], b4_d[r0:r0 + CH, :, :].transpose([1, 0, 2]))
            xk4 = opch.tile([4, CH * D], BF16, tag="xk4")
            nc.sync.dma_start(xk4[:], x4_d[r0:r0 + CH, :, :].transpose([1, 0, 2]))
            # hoist the PE-path scale tiles to the chunk head so the
            # identity-matmul adds never stall the TensorE FIFO on DVE
            ts_pre = {}
            for mloc in range(CH):
                m = r0 + mloc
                if m % 4 == 3:
                    ts = stage_pool.tile([128, n], F32, tag="ts",
                                         name=f"ts{mloc}")
                    for nb in range(nbs):
                        sl = slice(nb * 128, (nb + 1) * 128)
                        nc.vector.tensor_scalar_mul(
                            ts[:, sl], B_sb[:, sl],
                            aT_sb[:, nb * mc + m: nb * mc + m + 1])
                    ts_pre[mloc] = ts
            for mloc in range(CH):
                m = r0 + mloc
                pe_path = (m % 4 == 3)
                pos = [psum_main.tile([128, nw], F32, tag="pom", name=f"pom{h}")
                       for h in range(nh)]
                if pe_path:
                    # DVE did the cheap 2x-mode scale up front; the add runs
                    # on TensorE as an exact fp32 identity-matmul accumulate.
                    ts = ts_pre[mloc]
                    for nb in range(nbs):
                        g, go = (nb * 128) // nw, (nb * 128) % nw
                        sl = slice(nb * 128, (nb + 1) * 128)
                        nc.tensor.matmul(
                            pos[g][:, go:go + 128],
                            bk4[:, mloc * n + nb * 128: mloc * n + (nb + 1) * 128],
                            xk4[:, mloc * D:(mloc + 1) * D],
                            start=True, stop=False)
                        nc.tensor.matmul(
                            pos[g][:, go:go + 128], eye_sb[:], ts[:, sl],
                            start=False, stop=True)
                    ot = out_pool.tile([128, n], F32, tag="ot")
                    for h in range(nh):
                        nc.scalar.copy(ot[:, h * nw:(h + 1) * nw], pos[h][:])
                else:
                    for nb in range(nbs):
                        g, go = (nb * 128) // nw, (nb * 128) % nw
                        nc.tensor.matmul(
                            pos[g][:, go:go + 128],
                            bk4[:, mloc * n + nb * 128: mloc * n + (nb + 1) * 128],
                            xk4[:, mloc * D:(mloc + 1) * D],
                            start=True, stop=True)
                    # evacuate b*x from PSUM on the (otherwise idle) ScalarE
                    t2 = stage_pool.tile([128, n], F32, tag="t2")
                    for h in range(nh):
                        nc.scalar.copy(t2[:, h * nw:(h + 1) * nw], pos[h][:])
                    ot = out_pool.tile([128, n], F32, tag="ot")
                    for nb in range(nbs):
                        sl = slice(nb * 128, (nb + 1) * 128)
                        nc.vector.scalar_tensor_tensor(
                            out=ot[:, sl], in0=B_sb[:, sl],
                            scalar=aT_sb[:, nb * mc + m: nb * mc + m + 1],
                            in1=t2[:, sl], op0=ALU.mult, op1=ALU.add)
                slab = out_d[m * n:(m + 1) * n, :].rearrange(
                    "(nb p) d -> p nb d", p=128)
                dma_engines[m % 2].dma_start(slab, ot[:])


def build_program(mc=MC, n=N, repeat=1):
    nc = bacc.Bacc("TRN2", target_bir_lowering=False, debug=False,
                   num_devices=NCORES)
    B_d = nc.dram_tensor("b_in", [n, D], F32, kind="ExternalInput").ap()
    BT_d = nc.dram_tensor("bt_in", [D, n], F32, kind="ExternalInput").ap()
    xT_d = nc.dram_tensor("xt_in", [D, mc], F32, kind="ExternalInput").ap()
    b4_d = nc.dram_tensor("b4_scratch", [mc, 4, n], BF16).ap()
    x4_d = nc.dram_tensor("x4_in", [mc, 4, D], BF16,
                          kind="ExternalInput").ap()
    eye_d = nc.dram_tensor("eye_in", [128, 128], F32,
                           kind="ExternalInput").ap()
    out_d = nc.dram_tensor("out", [mc * n, D], F32, kind="ExternalOutput").ap()
    with tile.TileContext(nc) as tc:
        for _ in range(repeat):
            with ExitStack() as ctx:
                _body(ctx, tc, out_d, B_d, BT_d, xT_d, b4_d, x4_d, eye_d, mc, n)
    nc.compile()
    return nc


_NC_CACHE = None


def _get_nc():
    global _NC_CACHE
    if _NC_CACHE is None:
        _NC_CACHE = build_program()
    return _NC_CACHE


def make_in_maps(B, x):
    B = np.ascontiguousarray(np.asarray(B, dtype=np.float32))
    x = np.ascontiguousarray(np.asarray(x, dtype=np.float32))
    BT = np.ascontiguousarray(B.T)
    in_maps = []
    for c in range(NCORES):
        xs = np.ascontiguousarray(x[c * MC:(c + 1) * MC])
        import ml_dtypes
        xh = xs.astype(ml_dtypes.bfloat16)
        xl = (xs - xh.astype(np.float32)).astype(ml_dtypes.bfloat16)
        x4 = np.stack([xh, xh, xl, xl], axis=1)  # [mc, 4, D]
        in_maps.append({
            "b_in": B,
            "bt_in": BT,
            "xt_in": np.ascontiguousarray(xs.T),
            "x4_in": np.ascontiguousarray(x4),
            "eye_in": np.eye(128, dtype=np.float32),
        })
    return in_maps


def kernel(B, x):
    from concourse.bass_utils import run_bass_kernel_spmd
    nc = _get_nc()
    in_maps = make_in_maps(B, x)
    res = run_bass_kernel_spmd(nc, in_maps, list(range(NCORES)))
    outs = [np.asarray(res.results[c]["out"]).reshape(MC, N, D)
            for c in range(NCORES)]
    return np.concatenate(outs, axis=0)
